# revision 13
# baseline (speedup 1.0000x reference)
"""Trainium2 Bass kernel for nn_BaseAttention (B=4, H=16, S=2048, D=64, key-mask).

Strategy (8 NeuronCores, batch*head sharded, 8 heads per core; each core's 8
heads share one batch's mask):

* Host-side packing/layout (index gather + transposes only, no math):
  - The key mask is per-(batch, key) and masks ~half the keys with -1e4,
    whose exp underflows to exactly 0 in f32.  kernel() gathers the unmasked
    keys of K and V per batch and zero-pads to a common capacity cap
    (multiple of 128) — identical math, ~half the exp/matmul work.
  - Q is shipped pre-transposed as [2D, S] (Q^T duplicated into both
    partition halves), K as the pair-stacked transpose [128, T2*128] with
    row c*64+d, col u*128+p = K[(2u+c)*128+p, d].  So the device does no
    transposes at all, and mm1 can run k-tile pairs concurrently in the two
    row halves of the PE array (row tiling).
  - V' = [V | ones | 0-pad] -> [cap, 80]; the ones column gives the softmax
    denominator via mm2 (zero for padded keys).
  - The kernel stores out'^T = [64, S] (numerator/denominator already
    divided); the host transposes back to [S, 64].

* Per head on device:
  - mm1 (k-pair-major): scores S^T[t] = Kp[t] @ Q^T land in a hand-sliced
    6-slot PSUM ring ([128, 6, 512] f32); a k-tile pair (rows 0-63 / 64-127)
    fills 4 slots; one ScalarE ACTIVATE with a (possibly wrapped) 4-slot AP
    computes P^T = Exp(S^T/8) with N=2048, amortizing the ~350-cycle
    ACTIVATE overhead.  No max-subtraction: scores ~N(0,1) after the 1/8
    scale; padded keys give exp(0)=1 but their V' rows are all-zero.
  - mm2 window-major: acc[w] [80,512] f32 accumulates V'[t]^T P^T[t] over
    all t; 2 acc banks (PSUM = 6 + 2 = 8 banks).  P^T pair-chunks stay
    parked in SBUF.
  - Per window: drain acc [80,512] to SBUF f32.  Per head: broadcast the
    sums row across partitions (GpSimd) and divide (DVE), store out'^T.
* Emission is a flat software pipeline over (head, pair, q-half) chunks;
  head h's mm2/epilogue work is spread across head h+1's chunks.

Self-contained: hardcodes shapes; imports concourse from /opt/trn_rl_repo.
"""

import sys

if "/opt/trn_rl_repo" not in sys.path:
    sys.path.insert(0, "/opt/trn_rl_repo")

import numpy as np

import concourse.bass as bass
import concourse.mybir as mybir
import concourse.tile as tile
from concourse import bacc

F32 = mybir.dt.float32
BF16 = mybir.dt.bfloat16

N_CORES = 8
B, NH, S, D = 4, 16, 2048, 64
H = (B * NH) // N_CORES  # heads per core = 8
P = 128                  # partitions / k-tile size
W = 512                  # q window width (PSUM fp32 bank)
NW = S // W              # 4 windows
VC = 80                  # V' columns: 64 v + 1 ones + 15 zero pad (16-aligned)
NSLOT = 6                # score ring slots ([128, 512] f32, 1 bank each)
SCALE = 1.0 / 8.0        # 1/sqrt(D)


def emit_core_program(ctx, nc, tc, T, q_h, k_h, v_h, out_h):
    """Per-core program.

    q: [H, 2D, S] (Q^T, both halves); k: [H, 128, T2*128] (pair-stacked K^T);
    v: [H, T*128, 80]; out: [H, D, S] (= out'^T).
    """
    T2 = (T + 1) // 2
    pool = lambda *a, **kw: ctx.enter_context(tc.tile_pool(*a, **kw))
    ld = pool(name="ld", bufs=3)              # qT/kT/V' staging (bf16)
    ppool = pool(name="p", bufs=T2 + 3)       # P^T pair chunks [128, 2, 2048]
    accs_pool = pool(name="accs", bufs=2)     # drained accumulators (f32)
    rep_pool = pool(name="rep", bufs=2)       # broadcast denominators
    ost_pool = pool(name="ost", bufs=2)       # output staging f32

    st_pool = pool(name="stp", bufs=1, space="PSUM")
    st = st_pool.tile([P, NSLOT, W], F32, name="st")      # 6-bank score ring
    acc_pool = pool(name="acc", bufs=2, space="PSUM")     # 2 banks

    def emit_head_load(h):
        qT = ld.tile([P, S], BF16, tag="qT", name=f"qT_{h}")
        nc.gpsimd.dma_start(out=qT, in_=q_h[h])
        kT = ld.tile([P, T2 * P], BF16, tag="kT", name=f"kT_{h}")
        nc.gpsimd.dma_start(out=kT, in_=k_h[h])
        v_sb = ld.tile([P, T, VC], BF16, tag="v_sb", name=f"v_sb_{h}")
        nc.gpsimd.dma_start(out=v_sb, in_=v_h[h].rearrange("(t p) c -> p t c", p=P))
        return qT, kT, v_sb

    slot_ctr = [0]

    def st_ap(s0, nslots):
        # AP over ring slots s0..s0+nslots-1 (mod NSLOT), as [128, n/2, 2, W]
        base = st[:, 0, :]
        pdim = st.ap[0]
        ap = [pdim]
        if nslots == 4:
            delta = (s0 + 2) % NSLOT - s0
            ap.append([delta * W, 2])
        ap += [[W, 2], [1, W]]
        return bass.AP(tensor=st.tensor, offset=st.offset + s0 * W, ap=ap)

    def emit_chunk(h, u, c2):
        # one q-half of k-tile pair u: 2 or 4 matmuls + one exp ACTIVATE
        qT, kT, _ = heads[h]
        members = [c for c in range(2) if 2 * u + c < T]
        n = 2 * len(members)
        s0 = slot_ctr[0] % NSLOT
        slot_ctr[0] += n
        for c in range(2):  # interleave halves for row-tiling concurrency
            for m in members:
                lo = m * D
                nc.tensor.matmul(
                    st[:, (s0 + 2 * m + c) % NSLOT, :],
                    lhsT=kT[lo : lo + D, u * P : (u + 1) * P],
                    rhs=qT[lo : lo + D, c2 * 1024 + c * W : c2 * 1024 + (c + 1) * W],
                    start=True,
                    stop=True,
                )
        if c2 == 0:
            pTs[(h, u)] = ppool.tile([P, 2, S], BF16, tag="pT", name=f"pT_{h}_{u}")
        pT = pTs[(h, u)]
        pr = pT.rearrange("p a (b w) -> p a b w", w=W)
        if len(members) == 2:
            out_ap = pr[:, :, 2 * c2 : 2 * c2 + 2, :]
        else:
            out_ap = pr[:, 0, 2 * c2 : 2 * c2 + 2, :]
        nc.scalar.activation(
            out=out_ap,
            in_=st_ap(s0, n),
            func=mybir.ActivationFunctionType.Exp,
            scale=SCALE,
        )

    def mm2_window_thunk(h, w):
        def f():
            v_sb = heads[h][2]
            acc = acc_pool.tile([VC, W], F32, tag="acc", name=f"acc_{h}_{w}")
            accs_by_hw[(h, w)] = acc
            for t in range(T):
                nc.tensor.matmul(
                    acc,
                    lhsT=v_sb[:, t, :],
                    rhs=pTs[(h, t // 2)][:, t % 2, w * W : (w + 1) * W],
                    start=(t == 0),
                    stop=(t == T - 1),
                )
            if w == NW - 1:
                for u in range((T + 1) // 2):
                    del pTs[(h, u)]
        return f

    def drain_window_thunk(h, w):
        def f():
            acc = accs_by_hw.pop((h, w))
            if w == 0:
                accs_by_head[h] = accs_pool.tile(
                    [VC, NW, W], F32, tag="accs", name=f"accs_{h}"
                )
            nc.vector.tensor_copy(accs_by_head[h][:, w, :], acc)
        return f

    def epilogue_thunk(h):
        def f():
            accs = accs_by_head.pop(h)
            rec = rep_pool.tile([1, NW, W], F32, tag="rec")
            nc.vector.reciprocal(rec, accs[D : D + 1, :, :])
            rep = rep_pool.tile([D, NW * W], F32, tag="rep")
            nc.gpsimd.partition_broadcast(rep, rec, channels=D)
            ost = ost_pool.tile([D, NW * W], F32, tag="ost")
            nc.vector.tensor_mul(
                ost, accs.rearrange("c nw w -> c (nw w)")[0:D, :], rep
            )
            nc.sync.dma_start(out=out_h[h], in_=ost)
        return f

    units = [(h, u, c2) for h in range(H) for u in range(T2) for c2 in range(2)]
    heads = {0: emit_head_load(0)}
    pTs = {}
    accs_by_hw = {}
    accs_by_head = {}
    work = []

    for i, (h, u, c2) in enumerate(units):
        if u == 0 and c2 == 0 and h + 1 < H:
            heads[h + 1] = emit_head_load(h + 1)
        if u == 1 and c2 == 0 and h >= 2:
            del heads[h - 2]
        emit_chunk(h, u, c2)
        if u == T2 - 1 and c2 == 1:  # head's chunks all emitted: queue work
            for w in range(NW):
                work.append(mm2_window_thunk(h, w))
                work.append(drain_window_thunk(h, w))
            work.append(epilogue_thunk(h))
        # spread queued work across the next head's chunks
        npop = 1 if 2 * T2 >= 9 else -(-9 // (2 * T2))
        for _ in range(npop):
            if work:
                work.pop(0)()
    while work:
        work.pop(0)()


def build_nc(T):
    T2 = (T + 1) // 2
    nc = bacc.Bacc("TRN2", target_bir_lowering=False, debug=False, num_devices=N_CORES)
    q = nc.declare_dram_parameter("q", [H, 2 * D, S], F32, isOutput=False)
    k = nc.declare_dram_parameter("k", [H, P, T2 * P], F32, isOutput=False)
    v = nc.declare_dram_parameter("v", [H, T * P, VC], F32, isOutput=False)
    out = nc.declare_dram_parameter("out", [H, D, S], F32, isOutput=True)
    from contextlib import ExitStack

    with tile.TileContext(nc) as tc, ExitStack() as ctx:
        emit_core_program(ctx, nc, tc, T, q.ap(), k.ap(), v.ap(), out.ap())
    nc.compile()
    return nc


_NC_CACHE = {}


def get_nc(T):
    if T not in _NC_CACHE:
        _NC_CACHE[T] = build_nc(T)
    return _NC_CACHE[T]


def make_in_maps(q, k, v, mask):
    """Pack unmasked keys per batch; build device layouts; shard 8 cores."""
    qf = np.asarray(q, dtype=np.float32)
    kf = np.asarray(k, dtype=np.float32)
    vf = np.asarray(v, dtype=np.float32)
    mf = np.asarray(mask, dtype=np.int32).reshape(B, S)

    idxs = [np.flatnonzero(mf[b] == 0) for b in range(B)]
    maxcnt = max(len(ix) for ix in idxs)
    cap = min(S, max(P, -(-maxcnt // P) * P))
    T = cap // P
    T2 = (T + 1) // 2

    kp = np.zeros((B, NH, T2 * 2 * P, D), dtype=np.float32)
    vp = np.zeros((B, NH, cap, VC), dtype=np.float32)
    for b in range(B):
        n = len(idxs[b])
        kp[b, :, :n, :] = kf[b][:, idxs[b], :]
        vp[b, :, :n, :D] = vf[b][:, idxs[b], :]
        vp[b, :, :n, D] = 1.0

    # Q^T duplicated into both partition halves: [B,NH,S,D] -> [BNH, 2D, S]
    qT = qf.reshape(B * NH, S, D).transpose(0, 2, 1)
    qTd = np.concatenate([qT, qT], axis=1)
    # pair-stacked K^T: row c*64+d, col u*128+p = K[(2u+c)*128+p, d]
    k4 = kp.reshape(B * NH, T2, 2, P, D).transpose(0, 2, 4, 1, 3)
    kTd = k4.reshape(B * NH, 2 * D, T2 * P)
    vp = vp.reshape(B * NH, cap, VC)

    in_maps = []
    for c in range(N_CORES):
        lo = c * H
        in_maps.append(
            {
                "q": np.ascontiguousarray(qTd[lo : lo + H]),
                "k": np.ascontiguousarray(kTd[lo : lo + H]),
                "v": np.ascontiguousarray(vp[lo : lo + H]),
            }
        )
    return T, in_maps


def kernel(q, k, v, mask):
    from concourse.bass_utils import run_bass_kernel_spmd

    T, in_maps = make_in_maps(q, k, v, mask)
    nc = get_nc(T)
    try:
        res = run_bass_kernel_spmd(nc, in_maps, list(range(N_CORES))).results
    except Exception:
        # the axon execute path occasionally throws a transient INTERNAL
        # error right after a fresh NEFF compile; one retry clears it
        res = run_bass_kernel_spmd(nc, in_maps, list(range(N_CORES))).results
    # out is [H, D, S] per core (= out'^T): gather + host de-transpose
    out = np.concatenate([res[c]["out"] for c in range(N_CORES)], axis=0)
    return np.ascontiguousarray(out.transpose(0, 2, 1)).reshape(B, NH, S, D)


if __name__ == "__main__":
    nc = build_nc(int(sys.argv[1]) if len(sys.argv) > 1 else 9)
    print("built ok")


# revision 18
# speedup vs baseline: 1.2661x; 1.2661x over previous
"""Trainium2 Bass kernel for nn_BaseAttention (B=4, H=16, S=2048, D=64, key-mask).

Strategy (8 NeuronCores, batch*head sharded, 8 heads per core; each core's 8
heads share one batch's mask):

* Host-side packing/layout (index gather + transposes only, no math):
  - The key mask is per-(batch, key) and masks ~half the keys with -1e4,
    whose exp underflows to exactly 0 in f32.  kernel() gathers the unmasked
    keys of K and V per batch and zero-pads to a common capacity cap
    (multiple of 128) — identical math, ~half the exp/matmul work.
  - Q is shipped pre-transposed as [2D, S] (Q^T duplicated into both
    partition halves), K as the pair-stacked transpose [128, T2*128] with
    row c*64+d, col u*128+p = K[(2u+c)*128+p, d].  So the device does no
    transposes at all, and mm1 can run k-tile pairs concurrently in the two
    row halves of the PE array (row tiling).
  - V' = [V | ones | 0-pad] -> [cap, 80]; the ones column gives the softmax
    denominator via mm2 (zero for padded keys).
  - The kernel stores out'^T = [64, S] (numerator/denominator already
    divided); the host transposes back to [S, 64].

* Per head on device:
  - mm1 (k-pair-major): scores S^T[t] = Kp[t] @ Q^T land in a hand-sliced
    6-slot PSUM ring ([128, 6, 512] f32); a k-tile pair (rows 0-63 / 64-127)
    fills 4 slots; one ScalarE ACTIVATE with a (possibly wrapped) 4-slot AP
    computes P^T = Exp(S^T/8) with N=2048, amortizing the ~350-cycle
    ACTIVATE overhead.  No max-subtraction: scores ~N(0,1) after the 1/8
    scale; padded keys give exp(0)=1 but their V' rows are all-zero.
  - mm2 window-major: acc[w] [80,512] f32 accumulates V'[t]^T P^T[t] over
    all t; 2 acc banks (PSUM = 6 + 2 = 8 banks).  P^T pair-chunks stay
    parked in SBUF.
  - Per window: drain acc [80,512] to SBUF f32.  Per head: broadcast the
    sums row across partitions (GpSimd) and divide (DVE), store out'^T.
* Emission is a flat software pipeline over (head, pair, q-half) chunks;
  head h's mm2/epilogue work is spread across head h+1's chunks.

Self-contained: hardcodes shapes; imports concourse from /opt/trn_rl_repo.
"""

import sys

if "/opt/trn_rl_repo" not in sys.path:
    sys.path.insert(0, "/opt/trn_rl_repo")

import numpy as np

import concourse.bass as bass
import concourse.mybir as mybir
import concourse.tile as tile
from concourse import bacc

F32 = mybir.dt.float32
BF16 = mybir.dt.bfloat16

N_CORES = 8
B, NH, S, D = 4, 16, 2048, 64
H = (B * NH) // N_CORES  # heads per core = 8
P = 128                  # partitions / k-tile size
W = 512                  # q window width (PSUM fp32 bank)
NW = S // W              # 4 windows
VC = 128                 # V' columns: 64 v + 1 ones + 63 zero pad (FWL wants 128)
NSLOT = 6                # score ring slots ([128, 512] f32, 1 bank each)
SCALE = 1.0 / 8.0        # 1/sqrt(D)


def emit_core_program(ctx, nc, tc, T, q_h, k_h, v_h, out_h):
    """Per-core program.

    q: [H, 2D, S] (Q^T, both halves); k: [H, 128, T2*128] (pair-stacked K^T);
    v: [H, T*128, 80]; out: [H, D, S] (= out'^T).
    """
    T2 = (T + 1) // 2
    pool = lambda *a, **kw: ctx.enter_context(tc.tile_pool(*a, **kw))
    ld = pool(name="ld", bufs=3)              # qT/kT/V' staging (bf16)
    ppool = pool(name="p", bufs=T2 + 3)       # P^T pair chunks [128, 2, 2048]
    accs_pool = pool(name="accs", bufs=2)     # drained accumulators (f32)
    rep_pool = pool(name="rep", bufs=2)       # broadcast denominators
    ost_pool = pool(name="ost", bufs=2)       # output staging f32

    st_pool = pool(name="stp", bufs=1, space="PSUM")
    st = st_pool.tile([P, NSLOT, W], F32, name="st")      # 6-bank score ring
    acc_pool = pool(name="acc", bufs=1, space="PSUM")     # 2 banks

    def emit_head_load(h):
        qT = ld.tile([P, S], BF16, tag="qT", name=f"qT_{h}")
        nc.gpsimd.dma_start(out=qT, in_=q_h[h])
        kT = ld.tile([P, T2 * P], BF16, tag="kT", name=f"kT_{h}")
        nc.gpsimd.dma_start(out=kT, in_=k_h[h])
        v_sb = ld.tile([P, T, VC], BF16, tag="v_sb", name=f"v_sb_{h}")
        nc.gpsimd.dma_start(out=v_sb, in_=v_h[h].rearrange("(t p) c -> p t c", p=P))
        return qT, kT, v_sb

    slot_ctr = [0]

    def st_ap(s0, nslots):
        # AP over ring slots s0..s0+nslots-1 (mod NSLOT), as [128, n/2, 2, W]
        base = st[:, 0, :]
        pdim = st.ap[0]
        ap = [pdim]
        if nslots == 4:
            delta = (s0 + 2) % NSLOT - s0
            ap.append([delta * W, 2])
        ap += [[W, 2], [1, W]]
        return bass.AP(tensor=st.tensor, offset=st.offset + s0 * W, ap=ap)

    def emit_chunk(h, u, c2):
        # one q-half of k-tile pair u: 2 or 4 matmuls + one exp ACTIVATE
        qT, kT, _ = heads[h]
        members = [c for c in range(2) if 2 * u + c < T]
        n = 2 * len(members)
        s0 = slot_ctr[0] % NSLOT
        slot_ctr[0] += n
        for c in range(2):  # interleave halves for row-tiling concurrency
            for m in members:
                lo = m * D
                nc.tensor.matmul(
                    st[:, (s0 + 2 * m + c) % NSLOT, :],
                    lhsT=kT[lo : lo + D, u * P : (u + 1) * P],
                    rhs=qT[lo : lo + D, c2 * 1024 + c * W : c2 * 1024 + (c + 1) * W],
                    start=True,
                    stop=True,
                )
        if c2 == 0:
            pTs[(h, u)] = ppool.tile([P, 2, S], BF16, tag="pT", name=f"pT_{h}_{u}")
        pT = pTs[(h, u)]
        pr = pT.rearrange("p a (b w) -> p a b w", w=W)
        if len(members) == 2:
            out_ap = pr[:, :, 2 * c2 : 2 * c2 + 2, :]
        else:
            out_ap = pr[:, 0, 2 * c2 : 2 * c2 + 2, :]
        nc.scalar.activation(
            out=out_ap,
            in_=st_ap(s0, n),
            func=mybir.ActivationFunctionType.Exp,
            scale=SCALE,
        )

    def mm2_piece_thunk(h, p, t0, t1):
        # one weight-load of V'[t] serves both windows of pass p
        def f():
            v_sb = heads[h][2]
            if t0 == 0:
                acc = acc_pool.tile([VC, 2, W], F32, tag="acc", name=f"acc_{h}_{p}")
                accs_by_hw[(h, p)] = acc
            acc = accs_by_hw[(h, p)]
            for t in range(t0, t1):
                for j in range(2):
                    w = 2 * p + j
                    nc.tensor.matmul(
                        acc[:, j, :],
                        lhsT=v_sb[:, t, :],
                        rhs=pTs[(h, t // 2)][:, t % 2, w * W : (w + 1) * W],
                        start=(t == 0),
                        stop=(t == T - 1),
                    )
            if p == 1 and t1 == T:
                for u in range((T + 1) // 2):
                    del pTs[(h, u)]
        return f

    def drain_pass_thunk(h, p):
        def f():
            acc = accs_by_hw.pop((h, p))
            if p == 0:
                accs_by_head[h] = accs_pool.tile(
                    [VC, NW, W], F32, tag="accs", name=f"accs_{h}"
                )
            nc.vector.tensor_copy(accs_by_head[h][:, 2 * p : 2 * p + 2, :], acc)
        return f

    def epilogue_thunk(h):
        def f():
            accs = accs_by_head.pop(h)
            sums = rep_pool.tile([1, NW, W], F32, tag="sums")
            nc.vector.tensor_copy(sums, accs[D : D + 1, :, :])
            rec = rep_pool.tile([1, NW, W], F32, tag="rec")
            nc.vector.reciprocal_approx_fast(rec, sums)
            rep = rep_pool.tile([D, NW * W], F32, tag="rep")
            nc.gpsimd.partition_broadcast(rep, rec, channels=D)
            ost = ost_pool.tile([D, NW * W], F32, tag="ost")
            nc.vector.tensor_mul(
                ost, accs.rearrange("c nw w -> c (nw w)")[0:D, :], rep
            )
            nc.sync.dma_start(out=out_h[h], in_=ost)
        return f

    units = [(h, u, c2) for h in range(H) for u in range(T2) for c2 in range(2)]
    heads = {0: emit_head_load(0)}
    pTs = {}
    accs_by_hw = {}
    accs_by_head = {}
    work = []

    for i, (h, u, c2) in enumerate(units):
        if u == 0 and c2 == 0 and h + 1 < H:
            heads[h + 1] = emit_head_load(h + 1)
        if u == 1 and c2 == 0 and h >= 2:
            del heads[h - 2]
        emit_chunk(h, u, c2)
        if u == T2 - 1 and c2 == 1:  # head's chunks all emitted: queue work
            for p in range(2):
                for t0 in range(0, T, 3):
                    work.append(mm2_piece_thunk(h, p, t0, min(t0 + 3, T)))
                work.append(drain_pass_thunk(h, p))
            work.append(epilogue_thunk(h))
        # spread queued work across the next head's chunks
        nthunks = 2 * (-(-T // 3) + 1) + 1
        npop = -(-nthunks // (2 * T2))
        for _ in range(npop):
            if work:
                work.pop(0)()
    while work:
        work.pop(0)()


def build_nc(T):
    T2 = (T + 1) // 2
    nc = bacc.Bacc("TRN2", target_bir_lowering=False, debug=False, num_devices=N_CORES)
    q = nc.declare_dram_parameter("q", [H, 2 * D, S], F32, isOutput=False)
    k = nc.declare_dram_parameter("k", [H, P, T2 * P], F32, isOutput=False)
    v = nc.declare_dram_parameter("v", [H, T * P, VC], F32, isOutput=False)
    out = nc.declare_dram_parameter("out", [H, D, S], F32, isOutput=True)
    from contextlib import ExitStack

    with tile.TileContext(nc) as tc, ExitStack() as ctx:
        emit_core_program(ctx, nc, tc, T, q.ap(), k.ap(), v.ap(), out.ap())
    nc.compile()
    return nc


_NC_CACHE = {}


def get_nc(T):
    if T not in _NC_CACHE:
        _NC_CACHE[T] = build_nc(T)
    return _NC_CACHE[T]


def make_in_maps(q, k, v, mask):
    """Pack unmasked keys per batch; build device layouts; shard 8 cores."""
    qf = np.asarray(q, dtype=np.float32)
    kf = np.asarray(k, dtype=np.float32)
    vf = np.asarray(v, dtype=np.float32)
    mf = np.asarray(mask, dtype=np.int32).reshape(B, S)

    idxs = [np.flatnonzero(mf[b] == 0) for b in range(B)]
    maxcnt = max(len(ix) for ix in idxs)
    cap = min(S, max(P, -(-maxcnt // P) * P))
    T = cap // P
    T2 = (T + 1) // 2

    kp = np.zeros((B, NH, T2 * 2 * P, D), dtype=np.float32)
    vp = np.zeros((B, NH, cap, VC), dtype=np.float32)
    for b in range(B):
        n = len(idxs[b])
        kp[b, :, :n, :] = kf[b][:, idxs[b], :]
        vp[b, :, :n, :D] = vf[b][:, idxs[b], :]
        vp[b, :, :n, D] = 1.0

    # Q^T duplicated into both partition halves: [B,NH,S,D] -> [BNH, 2D, S]
    qT = qf.reshape(B * NH, S, D).transpose(0, 2, 1)
    qTd = np.concatenate([qT, qT], axis=1)
    # pair-stacked K^T: row c*64+d, col u*128+p = K[(2u+c)*128+p, d]
    k4 = kp.reshape(B * NH, T2, 2, P, D).transpose(0, 2, 4, 1, 3)
    kTd = k4.reshape(B * NH, 2 * D, T2 * P)
    vp = vp.reshape(B * NH, cap, VC)

    in_maps = []
    for c in range(N_CORES):
        lo = c * H
        in_maps.append(
            {
                "q": np.ascontiguousarray(qTd[lo : lo + H]),
                "k": np.ascontiguousarray(kTd[lo : lo + H]),
                "v": np.ascontiguousarray(vp[lo : lo + H]),
            }
        )
    return T, in_maps


def kernel(q, k, v, mask):
    from concourse.bass_utils import run_bass_kernel_spmd

    T, in_maps = make_in_maps(q, k, v, mask)
    nc = get_nc(T)
    try:
        res = run_bass_kernel_spmd(nc, in_maps, list(range(N_CORES))).results
    except Exception:
        # the axon execute path occasionally throws a transient INTERNAL
        # error right after a fresh NEFF compile; one retry clears it
        res = run_bass_kernel_spmd(nc, in_maps, list(range(N_CORES))).results
    # out is [H, D, S] per core (= out'^T): gather + host de-transpose
    out = np.concatenate([res[c]["out"] for c in range(N_CORES)], axis=0)
    return np.ascontiguousarray(out.transpose(0, 2, 1)).reshape(B, NH, S, D)


if __name__ == "__main__":
    nc = build_nc(int(sys.argv[1]) if len(sys.argv) > 1 else 9)
    print("built ok")
